# revision 35
# baseline (speedup 1.0000x reference)
"""Trainium2 Bass kernel for AttentionTwoLayers.

Computation (per batch element b):
  f        = features[b]                 # [C=1024, HW=256] (natural layout)
  featT    = Wv.T @ f                    # [I=512, HW=256]  (both operands natural!)
  state    = h[b] @ Ws + bs              # [I]
  hiddenT  = tanh(featT + (state+bv)[:,None])   # bias is per-partition -> fused in ACT
  logits   = Wl.T @ hiddenT              # [1, HW]
  scores   = softmax(logits)             # bl cancels in softmax
  weighted = f @ scores                  # [C]   (fused mul+accum on DVE)

Sharding: batch 128 -> 16 per core across 8 cores; weights replicated.
"""

import sys

for _p in ("/opt/trn_rl_repo",):
    if _p not in sys.path:
        sys.path.insert(0, _p)

import numpy as np

import concourse.bass as bass
import concourse.bacc as bacc
import concourse.mybir as mybir
from concourse.masks import make_identity
from concourse.tile import TileContext

F32 = mybir.dt.float32
F32R = mybir.dt.float32r

B, C, H, W = 128, 1024, 16, 16
HW = H * W          # 256
I = 512             # INT
L = 512             # LSTM
NCORES = 8
BLOC = B // NCORES  # 16 batch elements per core
CH = C // 128       # 8 contraction chunks for the main matmul
MI = I // 128       # 4 partition chunks of I
KL = L // 128       # 4 contraction chunks for the state matmul
NPAIR = BLOC // 2   # 8 pairs
GROUP = 4           # softmax batching group

MM_DT = F32R        # weights/hid matmul dtype (f32r = full-rate fp32)
BF16 = mybir.dt.bfloat16
X_DT = F32R         # features dtype
DEBUG = False       # add intermediate-dump outputs


def _mm(ap):
    return ap.bitcast(MM_DT) if MM_DT != F32 else ap


def build_nc():
    nc = bacc.Bacc("TRN2", target_bir_lowering=False, debug=False)

    f_d = nc.dram_tensor("features", [BLOC, C, HW], X_DT, kind="ExternalInput").ap()
    wv_d = nc.dram_tensor("Wv", [C, I], MM_DT, kind="ExternalInput").ap()
    ws_d = nc.dram_tensor("Ws", [L, I], F32, kind="ExternalInput").ap()
    ht_d = nc.dram_tensor("h_stateT", [L, BLOC], F32, kind="ExternalInput").ap()
    bvs_d = nc.dram_tensor("bvs", [I], F32, kind="ExternalInput").ap()
    wl_d = nc.dram_tensor("Wl", [I, 1], MM_DT, kind="ExternalInput").ap()
    wout_d = nc.dram_tensor("weighted", [BLOC, C], F32, kind="ExternalOutput").ap()
    sout_d = nc.dram_tensor("scores", [BLOC, HW], F32, kind="ExternalOutput").ap()

    Tanh = mybir.ActivationFunctionType.Tanh
    Exp = mybir.ActivationFunctionType.Exp
    MUL = mybir.AluOpType.mult

    with TileContext(nc) as tc:
        with (
            tc.tile_pool(name="consts", bufs=1) as consts,
            tc.tile_pool(name="xpool", bufs=6) as xpool,
            tc.tile_pool(name="hidpool", bufs=3) as hidpool,
            tc.tile_pool(name="stage", bufs=3) as stagepool,
            tc.tile_pool(name="lgpool", bufs=3) as lgpool,
            tc.tile_pool(name="smpool", bufs=8) as smpool,
            tc.tile_pool(name="bcpool", bufs=4) as bcpool,
            tc.tile_pool(name="scratch", bufs=3) as scratchpool,
            tc.tile_pool(name="outs", bufs=1) as outpool,
            tc.tile_pool(name="dscr", bufs=4, space="DRAM") as dram_pool,
            tc.tile_pool(name="pfeat", bufs=4, space="PSUM") as pfeat,
            tc.tile_pool(name="plg", bufs=2, space="PSUM") as plg,
            tc.tile_pool(name="pmisc", bufs=1, space="PSUM") as pmisc,
        ):
            f_r = f_d.rearrange("(p t) (ch cl) s -> p cl ch t s", t=2, cl=128)
            # Wv laid out per-mi so the first matmul group only needs 1/4 of it
            wv_r = wv_d.rearrange("(ch cl) (mi ii) -> mi cl ch ii", cl=128, ii=128)
            xs = [None] * NPAIR

            def load_x(p, split_first_chunk=False):
                x_sb = xpool.tile([128, CH, 2, HW], X_DT, name="x_sb")
                xs[p] = x_sb
                if split_first_chunk:
                    # land ch=0 first so the first matmul group starts sooner
                    nc.sync.dma_start(out=x_sb[:, 0:1, :, :], in_=f_r[p, :, 0:1, :, :])
                    for t in range(2):
                        nc.sync.dma_start(
                            out=x_sb[:, 1:, t, :], in_=f_r[p, :, 1:, t, :]
                        )
                else:
                    for t in range(2):
                        nc.sync.dma_start(out=x_sb[:, :, t, :], in_=f_r[p, :, :, t, :])

            # HAM warm-up: junk matmuls with no input deps keep the PE at full
            # clock while the first loads are in flight
            junk_sb = consts.tile([128, 512], mybir.dt.bfloat16)
            nc.gpsimd.memset(junk_sb, 0.0)
            junk_ps = pmisc.tile([128, 512], F32, tag="junkp", bufs=1)
            for _ in range(12):
                nc.tensor.matmul(
                    junk_ps, lhsT=junk_sb[:, :128], rhs=junk_sb, start=True, stop=True
                )

            # startup order: x0 arrives in 256KB ch-strips interleaved with
            # wv[mi] slices so the first matmul group streams as data lands
    
            x0 = xpool.tile([128, CH, 2, HW], X_DT, name="x_sb", tag="x_sb")
            xs[0] = x0
            nc.sync.dma_start(out=x0[:, 0:1, :, :], in_=f_r[0, :, 0:1, :, :])
            wv_sb = consts.tile([128, MI, CH, 128], MM_DT)
            nc.sync.dma_start(out=wv_sb[:, 0], in_=wv_r[0])
            nc.sync.dma_start(out=x0[:, 1:2, :, :], in_=f_r[0, :, 1:2, :, :])
            nc.sync.dma_start(out=x0[:, 2:3, :, :], in_=f_r[0, :, 2:3, :, :])
            for t in range(2):
                nc.sync.dma_start(out=x0[:, 3:, t, :], in_=f_r[0, :, 3:, t, :])
            ht_sb = consts.tile([128, KL, BLOC], F32)
            nc.sync.dma_start(
                out=ht_sb, in_=ht_d.rearrange("(kh kl) b -> kl kh b", kl=128)
            )
            wl_sb = consts.tile([128, MI], MM_DT)
            nc.sync.dma_start(
                out=wl_sb, in_=wl_d.rearrange("(mh ml) one -> ml (mh one)", ml=128)
            )
            bvs_sb = consts.tile([128, MI], F32)
            nc.sync.dma_start(
                out=bvs_sb, in_=bvs_d.rearrange("(mh ml) -> ml mh", ml=128)
            )
            ws_sb = consts.tile([128, KL, I], F32)
            ws_r = ws_d.rearrange("(kh kl) i -> kh kl i", kl=128)
            for kh in range(KL):
                nc.sync.dma_start(out=ws_sb[:, kh, :], in_=ws_r[kh])
            for mi in range(1, MI):
                nc.sync.dma_start(out=wv_sb[:, mi], in_=wv_r[mi])
            load_x(1)
            load_x(2)
            load_x(3)
            ident128 = consts.tile([128, 128], F32)
            make_identity(nc, ident128)

            sbias_sb = consts.tile([128, MI, BLOC], F32)
            wsum_sb = outpool.tile([128, CH, BLOC], F32)

            def emit_state():
                # state_out^T = Ws.T @ h^T + (bv + bs); emitted after pair 0's
                # matmuls so the PE doesn't stall on ws/ht at the queue head
                for mi in range(MI):
                    pso = pmisc.tile([128, BLOC], F32, tag="pm")
                    for kh in range(KL):
                        nc.tensor.matmul(
                            pso,
                            lhsT=ws_sb[:, kh, mi * 128 : (mi + 1) * 128],
                            rhs=ht_sb[:, kh, :],
                            start=(kh == 0),
                            stop=(kh == KL - 1),
                        )
                    nc.vector.tensor_scalar_add(
                        sbias_sb[:, mi, :], pso, bvs_sb[:, mi : mi + 1]
                    )

            def process(p, ts, last_unit):
                nt = len(ts)
                # ---- main matmuls ----
                pss = []
                for mi in range(MI):
                    ps = pfeat.tile([128, nt * HW], F32, name="ps", tag="ps")
                    pss.append(ps)
                    for ch in range(CH):
                        rhs = (
                            xs[p][:, ch, :, :]
                            if nt == 2
                            else xs[p][:, ch, ts[0], :]
                        )
                        nc.tensor.matmul(
                            ps,
                            lhsT=wv_sb[:, mi, ch, :],
                            rhs=rhs,
                            start=(ch == 0),
                            stop=(ch == CH - 1),
                        )
                    if p == 0 and mi == 0:
                        emit_state()
                    if p < 3 and nt == 2:
                        for _ in range(2):
                            nc.tensor.matmul(
                                junk_ps,
                                lhsT=junk_sb[:, :128],
                                rhs=junk_sb,
                                start=True,
                                stop=True,
                            )
                if p + 4 < NPAIR and ts[0] == 0:
                    load_x(p + 4)

                # ---- tanh + per-partition state bias ----
                hid_sb = hidpool.tile([128, MI, nt, HW], MM_DT, name="hid_sb", tag="hid")
                for mi in range(MI):
                    for tj, t in enumerate(ts):
                        nc.scalar.activation(
                            out=hid_sb[:, mi, tj, :],
                            in_=pss[mi][:, tj * HW : (tj + 1) * HW],
                            func=Tanh,
                            bias=sbias_sb[:, mi, 2 * p + t : 2 * p + t + 1],
                            scale=1.0,
                        )

                # ---- logits ----
                psl = plg.tile([1, nt * HW], F32, name="psl", tag="psl")
                for mi in range(MI):
                    nc.tensor.matmul(
                        psl,
                        lhsT=wl_sb[:, mi : mi + 1],
                        rhs=hid_sb[:, mi, :, :],
                        start=(mi == 0),
                        stop=(mi == MI - 1),
                    )

                # ---- softmax on partition 0 (logits bounded -> exp is safe
                # without max-subtraction, mathematically identical) ----
                es = smpool.tile([1, nt, HW], F32, tag="es", name="es")
                sums = smpool.tile([1, nt], F32, tag="sums", name="sums")
                if nt == 1:
                    nc.scalar.activation(out=es, in_=psl, func=Exp, accum_out=sums)
                else:
                    nc.scalar.activation(out=es, in_=psl, func=Exp)
                    nc.vector.reduce_sum(out=sums, in_=es, axis=mybir.AxisListType.X)
                rsum = smpool.tile([1, nt], F32, tag="rsum", name="rsum")
                nc.vector.reciprocal(out=rsum, in_=sums)
                scores_st = smpool.tile(
                    [1, nt, HW], F32, tag="scores_st", name="scores_st"
                )
                if nt == 1:
                    nc.vector.tensor_scalar_mul(scores_st, es, rsum)
                else:
                    nc.vector.scalar_tensor_tensor(
                        out=scores_st,
                        in0=es,
                        scalar=1.0,
                        in1=rsum.unsqueeze(2).to_broadcast([1, nt, HW]),
                        op0=MUL,
                        op1=MUL,
                    )
                r0 = 2 * p + ts[0]
                nc.sync.dma_start(out=sout_d[r0 : r0 + nt, :], in_=scores_st)

                # ---- pooling: broadcast scores to all partitions (gpsimd),
                # then fused multiply+accumulate per 128-channel chunk ----
                bcast = bcpool.tile([128, nt, HW], F32, name="bcast", tag="bc")
                nc.gpsimd.partition_broadcast(
                    bcast, scores_st.rearrange("one b s -> one (b s)")
                )
                if last_unit:
                    wT_half = outpool.tile(
                        [8, CH, 128], F32, tag="whalf", bufs=2, name="wT_half"
                    )
                for tj, t in enumerate(ts):
                    i_loc = 2 * p + t
                    for ch in range(CH):
                        sc1 = scratchpool.tile([128, HW], F32, name="sc1")
                        nc.vector.scalar_tensor_tensor(
                            out=sc1,
                            in0=xs[p][:, ch, t, :].bitcast(F32),
                            scalar=1.0,
                            in1=bcast[:, tj, :],
                            op0=MUL,
                            op1=MUL,
                            accum_out=wsum_sb[:, ch, i_loc : i_loc + 1],
                        )
                        if last_unit and t == ts[-1]:
                            # interleave the final per-ch transpose with the
                            # remaining pooling ops (PE is idle here)
                            pst = pmisc.tile([BLOC, 128], F32, tag="pm")
                            nc.tensor.transpose(
                                out=pst[:8, :],
                                in_=wsum_sb[:, ch, 8:16],
                                identity=ident128,
                            )
                            nc.scalar.copy(wT_half[:, ch, :], pst[:8, :])
                if last_unit:
                    nc.sync.dma_start(
                        out=wout_d[8:16, :],
                        in_=wT_half.rearrange("b ch cl -> b (ch cl)"),
                    )

                # ---- first-half weighted transpose mid-kernel ----
                if p == NPAIR // 2 - 1 and ts[-1] == 1:
                    wT_h0 = outpool.tile(
                        [8, CH, 128], F32, tag="whalf", bufs=2, name="wT_h0"
                    )
                    for ch in range(CH):
                        pst = pmisc.tile([BLOC, 128], F32, tag="pm")
                        nc.tensor.transpose(
                            out=pst[:8, :],
                            in_=wsum_sb[:, ch, 0:8],
                            identity=ident128,
                        )
                        nc.scalar.copy(wT_h0[:, ch, :], pst[:8, :])
                    nc.sync.dma_start(
                        out=wout_d[0:8, :],
                        in_=wT_h0.rearrange("b ch cl -> b (ch cl)"),
                    )

            for p in range(NPAIR - 1):
                process(p, (0, 1), False)
            process(NPAIR - 1, (0,), False)
            process(NPAIR - 1, (1,), True)

    nc.compile()
    return nc


_NC = None


def _get_nc():
    global _NC
    if _NC is None:
        _NC = build_nc()
    return _NC


def _round_fp32r(a):
    """Round fp32 to the PE's FP32r format (11-bit mantissa, half-to-even),
    matching walrus's fp32_to_fp32r. Required for tensors consumed by FP32r
    matmuls: the HW expects pre-rounded operands."""
    u = np.ascontiguousarray(a, dtype=np.float32).view(np.uint32)
    lsb = (u >> np.uint32(12)) & np.uint32(1)
    r = (u + np.uint32(0x7FF) + lsb) & np.uint32(0xFFFFF000)
    return r.view(np.float32)


def make_in_maps(features, h_state, Wv, bv, Ws, bs, Wl):
    feats = _round_fp32r(
        np.asarray(features, dtype=np.float32).reshape(B, C, HW)
    )
    hs = np.ascontiguousarray(np.asarray(h_state, dtype=np.float32))
    bvs = (np.asarray(bv, np.float32) + np.asarray(bs, np.float32)).astype(np.float32)
    common = {
        "Wv": _round_fp32r(np.asarray(Wv, dtype=np.float32)),
        "Ws": np.ascontiguousarray(np.asarray(Ws, dtype=np.float32)),
        "Wl": _round_fp32r(np.asarray(Wl, dtype=np.float32)),
        "bvs": bvs,
    }
    return [
        {
            "features": feats[r * BLOC : (r + 1) * BLOC],
            "h_stateT": np.ascontiguousarray(hs[r * BLOC : (r + 1) * BLOC].T),
            **common,
        }
        for r in range(NCORES)
    ]


def kernel(features, h_state, Wv, bv, Ws, bs, Wl, bl):
    from concourse.bass_utils import run_bass_kernel_spmd

    nc = _get_nc()
    in_maps = make_in_maps(features, h_state, Wv, bv, Ws, bs, Wl)
    res = run_bass_kernel_spmd(nc, in_maps, core_ids=list(range(NCORES))).results
    weighted = np.concatenate([res[r]["weighted"] for r in range(NCORES)], axis=0)
    scores = np.concatenate([res[r]["scores"] for r in range(NCORES)], axis=0)
    return weighted, scores.reshape(B, H, W)


# revision 36
# speedup vs baseline: 1.0197x; 1.0197x over previous
"""Trainium2 Bass kernel for AttentionTwoLayers.

Computation (per batch element b):
  f        = features[b]                 # [C=1024, HW=256] (natural layout)
  featT    = Wv.T @ f                    # [I=512, HW=256]  (both operands natural!)
  state    = h[b] @ Ws + bs              # [I]
  hiddenT  = tanh(featT + (state+bv)[:,None])   # bias is per-partition -> fused in ACT
  logits   = Wl.T @ hiddenT              # [1, HW]
  scores   = softmax(logits)             # bl cancels in softmax
  weighted = f @ scores                  # [C]   (fused mul+accum on DVE)

Sharding: batch 128 -> 16 per core across 8 cores; weights replicated.
"""

import sys

for _p in ("/opt/trn_rl_repo",):
    if _p not in sys.path:
        sys.path.insert(0, _p)

import numpy as np

import concourse.bass as bass
import concourse.bacc as bacc
import concourse.mybir as mybir
from concourse.masks import make_identity
from concourse.tile import TileContext

F32 = mybir.dt.float32
F32R = mybir.dt.float32r

B, C, H, W = 128, 1024, 16, 16
HW = H * W          # 256
I = 512             # INT
L = 512             # LSTM
NCORES = 8
BLOC = B // NCORES  # 16 batch elements per core
CH = C // 128       # 8 contraction chunks for the main matmul
MI = I // 128       # 4 partition chunks of I
KL = L // 128       # 4 contraction chunks for the state matmul
NPAIR = BLOC // 2   # 8 pairs
GROUP = 4           # softmax batching group

MM_DT = F32R        # weights/hid matmul dtype (f32r = full-rate fp32)
BF16 = mybir.dt.bfloat16
X_DT = F32R         # features dtype
DEBUG = False       # add intermediate-dump outputs


def _mm(ap):
    return ap.bitcast(MM_DT) if MM_DT != F32 else ap


def build_nc():
    nc = bacc.Bacc("TRN2", target_bir_lowering=False, debug=False)

    f_d = nc.dram_tensor("features", [BLOC, C, HW], X_DT, kind="ExternalInput").ap()
    wv_d = nc.dram_tensor("Wv", [C, I], MM_DT, kind="ExternalInput").ap()
    ws_d = nc.dram_tensor("Ws", [L, I], F32, kind="ExternalInput").ap()
    ht_d = nc.dram_tensor("h_stateT", [L, BLOC], F32, kind="ExternalInput").ap()
    bvs_d = nc.dram_tensor("bvs", [I], F32, kind="ExternalInput").ap()
    wl_d = nc.dram_tensor("Wl", [I, 1], MM_DT, kind="ExternalInput").ap()
    wout_d = nc.dram_tensor("weighted", [BLOC, C], F32, kind="ExternalOutput").ap()
    sout_d = nc.dram_tensor("scores", [BLOC, HW], F32, kind="ExternalOutput").ap()

    Tanh = mybir.ActivationFunctionType.Tanh
    Exp = mybir.ActivationFunctionType.Exp
    MUL = mybir.AluOpType.mult

    with TileContext(nc) as tc:
        with (
            tc.tile_pool(name="consts", bufs=1) as consts,
            tc.tile_pool(name="xpool", bufs=6) as xpool,
            tc.tile_pool(name="hidpool", bufs=3) as hidpool,
            tc.tile_pool(name="stage", bufs=3) as stagepool,
            tc.tile_pool(name="lgpool", bufs=3) as lgpool,
            tc.tile_pool(name="smpool", bufs=8) as smpool,
            tc.tile_pool(name="bcpool", bufs=4) as bcpool,
            tc.tile_pool(name="scratch", bufs=3) as scratchpool,
            tc.tile_pool(name="outs", bufs=1) as outpool,
            tc.tile_pool(name="dscr", bufs=4, space="DRAM") as dram_pool,
            tc.tile_pool(name="pfeat", bufs=4, space="PSUM") as pfeat,
            tc.tile_pool(name="plg", bufs=2, space="PSUM") as plg,
            tc.tile_pool(name="pmisc", bufs=1, space="PSUM") as pmisc,
        ):
            f_r = f_d.rearrange("(p t) (ch cl) s -> p cl ch t s", t=2, cl=128)
            # Wv laid out per-mi so the first matmul group only needs 1/4 of it
            wv_r = wv_d.rearrange("(ch cl) (mi ii) -> mi cl ch ii", cl=128, ii=128)
            xs = [None] * NPAIR

            def load_x(p, split_first_chunk=False):
                x_sb = xpool.tile([128, CH, 2, HW], X_DT, name="x_sb")
                xs[p] = x_sb
                if split_first_chunk:
                    # land ch=0 first so the first matmul group starts sooner
                    nc.sync.dma_start(out=x_sb[:, 0:1, :, :], in_=f_r[p, :, 0:1, :, :])
                    for t in range(2):
                        nc.sync.dma_start(
                            out=x_sb[:, 1:, t, :], in_=f_r[p, :, 1:, t, :]
                        )
                else:
                    for t in range(2):
                        nc.sync.dma_start(out=x_sb[:, :, t, :], in_=f_r[p, :, :, t, :])

            # HAM warm-up: junk matmuls with no input deps keep the PE at full
            # clock while the first loads are in flight
            junk_sb = consts.tile([128, 512], mybir.dt.bfloat16)
            nc.gpsimd.memset(junk_sb, 0.0)
            junk_ps = pmisc.tile([128, 512], F32, tag="junkp", bufs=1)
            for _ in range(12):
                nc.tensor.matmul(
                    junk_ps, lhsT=junk_sb[:, :128], rhs=junk_sb, start=True, stop=True
                )

            # startup order: x0 arrives in 256KB ch-strips interleaved with
            # wv[mi] slices so the first matmul group streams as data lands
    
            x0 = xpool.tile([128, CH, 2, HW], X_DT, name="x_sb", tag="x_sb")
            xs[0] = x0
            nc.sync.dma_start(out=x0[:, 0:1, :, :], in_=f_r[0, :, 0:1, :, :])
            wv_sb = consts.tile([128, MI, CH, 128], MM_DT)
            nc.sync.dma_start(out=wv_sb[:, 0], in_=wv_r[0])
            nc.sync.dma_start(out=x0[:, 1:2, :, :], in_=f_r[0, :, 1:2, :, :])
            nc.sync.dma_start(out=x0[:, 2:3, :, :], in_=f_r[0, :, 2:3, :, :])
            for t in range(2):
                nc.sync.dma_start(out=x0[:, 3:, t, :], in_=f_r[0, :, 3:, t, :])
            ht_sb = consts.tile([128, KL, BLOC], F32)
            nc.sync.dma_start(
                out=ht_sb, in_=ht_d.rearrange("(kh kl) b -> kl kh b", kl=128)
            )
            wl_sb = consts.tile([128, MI], MM_DT)
            nc.sync.dma_start(
                out=wl_sb, in_=wl_d.rearrange("(mh ml) one -> ml (mh one)", ml=128)
            )
            bvs_sb = consts.tile([128, MI], F32)
            nc.sync.dma_start(
                out=bvs_sb, in_=bvs_d.rearrange("(mh ml) -> ml mh", ml=128)
            )
            ws_sb = consts.tile([128, KL, I], F32)
            ws_r = ws_d.rearrange("(kh kl) i -> kh kl i", kl=128)
            for kh in range(KL):
                nc.sync.dma_start(out=ws_sb[:, kh, :], in_=ws_r[kh])
            for mi in range(1, MI):
                nc.sync.dma_start(out=wv_sb[:, mi], in_=wv_r[mi])
            load_x(1)
            load_x(2)
            load_x(3)
            ident128 = consts.tile([128, 128], F32)
            make_identity(nc, ident128)

            sbias_sb = consts.tile([128, MI, BLOC], F32)
            wsum_sb = outpool.tile([128, CH, BLOC], F32)

            def emit_state():
                # state_out^T = Ws.T @ h^T + (bv + bs); emitted after pair 0's
                # matmuls so the PE doesn't stall on ws/ht at the queue head
                for mi in range(MI):
                    pso = pmisc.tile([128, BLOC], F32, tag="pm")
                    for kh in range(KL):
                        nc.tensor.matmul(
                            pso,
                            lhsT=ws_sb[:, kh, mi * 128 : (mi + 1) * 128],
                            rhs=ht_sb[:, kh, :],
                            start=(kh == 0),
                            stop=(kh == KL - 1),
                        )
                    nc.vector.tensor_scalar_add(
                        sbias_sb[:, mi, :], pso, bvs_sb[:, mi : mi + 1]
                    )

            def process(p, ts, last_unit):
                nt = len(ts)
                # ---- main matmuls ----
                pss = []
                for mi in range(MI):
                    ps = pfeat.tile([128, nt * HW], F32, name="ps", tag="ps")
                    pss.append(ps)
                    for ch in range(CH):
                        rhs = (
                            xs[p][:, ch, :, :]
                            if nt == 2
                            else xs[p][:, ch, ts[0], :]
                        )
                        nc.tensor.matmul(
                            ps,
                            lhsT=wv_sb[:, mi, ch, :],
                            rhs=rhs,
                            start=(ch == 0),
                            stop=(ch == CH - 1),
                        )
                    if p == 0 and mi == 0:
                        emit_state()
                    if p < 3 and nt == 2:
                        for _ in range(3):
                            nc.tensor.matmul(
                                junk_ps,
                                lhsT=junk_sb[:, :128],
                                rhs=junk_sb,
                                start=True,
                                stop=True,
                            )
                if p + 4 < NPAIR and ts[0] == 0:
                    load_x(p + 4)

                # ---- tanh + per-partition state bias ----
                hid_sb = hidpool.tile([128, MI, nt, HW], MM_DT, name="hid_sb", tag="hid")
                for mi in range(MI):
                    for tj, t in enumerate(ts):
                        nc.scalar.activation(
                            out=hid_sb[:, mi, tj, :],
                            in_=pss[mi][:, tj * HW : (tj + 1) * HW],
                            func=Tanh,
                            bias=sbias_sb[:, mi, 2 * p + t : 2 * p + t + 1],
                            scale=1.0,
                        )

                # ---- logits ----
                psl = plg.tile([1, nt * HW], F32, name="psl", tag="psl")
                for mi in range(MI):
                    nc.tensor.matmul(
                        psl,
                        lhsT=wl_sb[:, mi : mi + 1],
                        rhs=hid_sb[:, mi, :, :],
                        start=(mi == 0),
                        stop=(mi == MI - 1),
                    )

                # ---- softmax on partition 0 (logits bounded -> exp is safe
                # without max-subtraction, mathematically identical) ----
                es = smpool.tile([1, nt, HW], F32, tag="es", name="es")
                sums = smpool.tile([1, nt], F32, tag="sums", name="sums")
                if nt == 1:
                    nc.scalar.activation(out=es, in_=psl, func=Exp, accum_out=sums)
                else:
                    nc.scalar.activation(out=es, in_=psl, func=Exp)
                    nc.vector.reduce_sum(out=sums, in_=es, axis=mybir.AxisListType.X)
                rsum = smpool.tile([1, nt], F32, tag="rsum", name="rsum")
                nc.vector.reciprocal(out=rsum, in_=sums)
                scores_st = smpool.tile(
                    [1, nt, HW], F32, tag="scores_st", name="scores_st"
                )
                if nt == 1:
                    nc.vector.tensor_scalar_mul(scores_st, es, rsum)
                else:
                    nc.vector.scalar_tensor_tensor(
                        out=scores_st,
                        in0=es,
                        scalar=1.0,
                        in1=rsum.unsqueeze(2).to_broadcast([1, nt, HW]),
                        op0=MUL,
                        op1=MUL,
                    )
                r0 = 2 * p + ts[0]
                nc.sync.dma_start(out=sout_d[r0 : r0 + nt, :], in_=scores_st)

                # ---- pooling: broadcast scores to all partitions (gpsimd),
                # then fused multiply+accumulate per 128-channel chunk ----
                bcast = bcpool.tile([128, nt, HW], F32, name="bcast", tag="bc")
                nc.gpsimd.partition_broadcast(
                    bcast, scores_st.rearrange("one b s -> one (b s)")
                )
                if last_unit:
                    wT_half = outpool.tile(
                        [8, CH, 128], F32, tag="whalf", bufs=2, name="wT_half"
                    )
                for tj, t in enumerate(ts):
                    i_loc = 2 * p + t
                    for ch in range(CH):
                        sc1 = scratchpool.tile([128, HW], F32, name="sc1")
                        nc.vector.scalar_tensor_tensor(
                            out=sc1,
                            in0=xs[p][:, ch, t, :].bitcast(F32),
                            scalar=1.0,
                            in1=bcast[:, tj, :],
                            op0=MUL,
                            op1=MUL,
                            accum_out=wsum_sb[:, ch, i_loc : i_loc + 1],
                        )
                        if last_unit and t == ts[-1]:
                            # interleave the final per-ch transpose with the
                            # remaining pooling ops (PE is idle here)
                            pst = pmisc.tile([BLOC, 128], F32, tag="pm")
                            nc.tensor.transpose(
                                out=pst[:8, :],
                                in_=wsum_sb[:, ch, 8:16],
                                identity=ident128,
                            )
                            nc.scalar.copy(wT_half[:, ch, :], pst[:8, :])
                if last_unit:
                    nc.sync.dma_start(
                        out=wout_d[8:16, :],
                        in_=wT_half.rearrange("b ch cl -> b (ch cl)"),
                    )

                # ---- first-half weighted transpose mid-kernel ----
                if p == NPAIR // 2 - 1 and ts[-1] == 1:
                    wT_h0 = outpool.tile(
                        [8, CH, 128], F32, tag="whalf", bufs=2, name="wT_h0"
                    )
                    for ch in range(CH):
                        pst = pmisc.tile([BLOC, 128], F32, tag="pm")
                        nc.tensor.transpose(
                            out=pst[:8, :],
                            in_=wsum_sb[:, ch, 0:8],
                            identity=ident128,
                        )
                        nc.scalar.copy(wT_h0[:, ch, :], pst[:8, :])
                    nc.sync.dma_start(
                        out=wout_d[0:8, :],
                        in_=wT_h0.rearrange("b ch cl -> b (ch cl)"),
                    )

            for p in range(NPAIR - 1):
                process(p, (0, 1), False)
            process(NPAIR - 1, (0,), False)
            process(NPAIR - 1, (1,), True)

    nc.compile()
    return nc


_NC = None


def _get_nc():
    global _NC
    if _NC is None:
        _NC = build_nc()
    return _NC


def _round_fp32r(a):
    """Round fp32 to the PE's FP32r format (11-bit mantissa, half-to-even),
    matching walrus's fp32_to_fp32r. Required for tensors consumed by FP32r
    matmuls: the HW expects pre-rounded operands."""
    u = np.ascontiguousarray(a, dtype=np.float32).view(np.uint32)
    lsb = (u >> np.uint32(12)) & np.uint32(1)
    r = (u + np.uint32(0x7FF) + lsb) & np.uint32(0xFFFFF000)
    return r.view(np.float32)


def make_in_maps(features, h_state, Wv, bv, Ws, bs, Wl):
    feats = _round_fp32r(
        np.asarray(features, dtype=np.float32).reshape(B, C, HW)
    )
    hs = np.ascontiguousarray(np.asarray(h_state, dtype=np.float32))
    bvs = (np.asarray(bv, np.float32) + np.asarray(bs, np.float32)).astype(np.float32)
    common = {
        "Wv": _round_fp32r(np.asarray(Wv, dtype=np.float32)),
        "Ws": np.ascontiguousarray(np.asarray(Ws, dtype=np.float32)),
        "Wl": _round_fp32r(np.asarray(Wl, dtype=np.float32)),
        "bvs": bvs,
    }
    return [
        {
            "features": feats[r * BLOC : (r + 1) * BLOC],
            "h_stateT": np.ascontiguousarray(hs[r * BLOC : (r + 1) * BLOC].T),
            **common,
        }
        for r in range(NCORES)
    ]


def kernel(features, h_state, Wv, bv, Ws, bs, Wl, bl):
    from concourse.bass_utils import run_bass_kernel_spmd

    nc = _get_nc()
    in_maps = make_in_maps(features, h_state, Wv, bv, Ws, bs, Wl)
    res = run_bass_kernel_spmd(nc, in_maps, core_ids=list(range(NCORES))).results
    weighted = np.concatenate([res[r]["weighted"] for r in range(NCORES)], axis=0)
    scores = np.concatenate([res[r]["scores"] for r in range(NCORES)], axis=0)
    return weighted, scores.reshape(B, H, W)


# revision 37
# speedup vs baseline: 1.0336x; 1.0136x over previous
"""Trainium2 Bass kernel for AttentionTwoLayers.

Computation (per batch element b):
  f        = features[b]                 # [C=1024, HW=256] (natural layout)
  featT    = Wv.T @ f                    # [I=512, HW=256]  (both operands natural!)
  state    = h[b] @ Ws + bs              # [I]
  hiddenT  = tanh(featT + (state+bv)[:,None])   # bias is per-partition -> fused in ACT
  logits   = Wl.T @ hiddenT              # [1, HW]
  scores   = softmax(logits)             # bl cancels in softmax
  weighted = f @ scores                  # [C]   (fused mul+accum on DVE)

Sharding: batch 128 -> 16 per core across 8 cores; weights replicated.
"""

import sys

for _p in ("/opt/trn_rl_repo",):
    if _p not in sys.path:
        sys.path.insert(0, _p)

import numpy as np

import concourse.bass as bass
import concourse.bacc as bacc
import concourse.mybir as mybir
from concourse.masks import make_identity
from concourse.tile import TileContext

F32 = mybir.dt.float32
F32R = mybir.dt.float32r

B, C, H, W = 128, 1024, 16, 16
HW = H * W          # 256
I = 512             # INT
L = 512             # LSTM
NCORES = 8
BLOC = B // NCORES  # 16 batch elements per core
CH = C // 128       # 8 contraction chunks for the main matmul
MI = I // 128       # 4 partition chunks of I
KL = L // 128       # 4 contraction chunks for the state matmul
NPAIR = BLOC // 2   # 8 pairs
GROUP = 4           # softmax batching group

MM_DT = F32R        # weights/hid matmul dtype (f32r = full-rate fp32)
BF16 = mybir.dt.bfloat16
X_DT = F32R         # features dtype
DEBUG = False       # add intermediate-dump outputs


def _mm(ap):
    return ap.bitcast(MM_DT) if MM_DT != F32 else ap


def build_nc():
    nc = bacc.Bacc("TRN2", target_bir_lowering=False, debug=False)

    f_d = nc.dram_tensor("features", [BLOC, C, HW], X_DT, kind="ExternalInput").ap()
    wv_d = nc.dram_tensor("Wv", [C, I], MM_DT, kind="ExternalInput").ap()
    ws_d = nc.dram_tensor("Ws", [L, I], F32, kind="ExternalInput").ap()
    ht_d = nc.dram_tensor("h_stateT", [L, BLOC], F32, kind="ExternalInput").ap()
    bvs_d = nc.dram_tensor("bvs", [I], F32, kind="ExternalInput").ap()
    wl_d = nc.dram_tensor("Wl", [I, 1], MM_DT, kind="ExternalInput").ap()
    wout_d = nc.dram_tensor("weighted", [BLOC, C], F32, kind="ExternalOutput").ap()
    sout_d = nc.dram_tensor("scores", [BLOC, HW], F32, kind="ExternalOutput").ap()

    Tanh = mybir.ActivationFunctionType.Tanh
    Exp = mybir.ActivationFunctionType.Exp
    MUL = mybir.AluOpType.mult

    with TileContext(nc) as tc:
        with (
            tc.tile_pool(name="consts", bufs=1) as consts,
            tc.tile_pool(name="xpool", bufs=6) as xpool,
            tc.tile_pool(name="hidpool", bufs=3) as hidpool,
            tc.tile_pool(name="stage", bufs=3) as stagepool,
            tc.tile_pool(name="lgpool", bufs=3) as lgpool,
            tc.tile_pool(name="smpool", bufs=8) as smpool,
            tc.tile_pool(name="bcpool", bufs=4) as bcpool,
            tc.tile_pool(name="scratch", bufs=3) as scratchpool,
            tc.tile_pool(name="outs", bufs=1) as outpool,
            tc.tile_pool(name="dscr", bufs=4, space="DRAM") as dram_pool,
            tc.tile_pool(name="pfeat", bufs=5, space="PSUM") as pfeat,
            tc.tile_pool(name="plg", bufs=1, space="PSUM") as plg,
            tc.tile_pool(name="pmisc", bufs=1, space="PSUM") as pmisc,
        ):
            f_r = f_d.rearrange("(p t) (ch cl) s -> p cl ch t s", t=2, cl=128)
            # Wv laid out per-mi so the first matmul group only needs 1/4 of it
            wv_r = wv_d.rearrange("(ch cl) (mi ii) -> mi cl ch ii", cl=128, ii=128)
            xs = [None] * NPAIR

            def load_x(p, split_first_chunk=False):
                x_sb = xpool.tile([128, CH, 2, HW], X_DT, name="x_sb")
                xs[p] = x_sb
                if split_first_chunk:
                    # land ch=0 first so the first matmul group starts sooner
                    nc.sync.dma_start(out=x_sb[:, 0:1, :, :], in_=f_r[p, :, 0:1, :, :])
                    for t in range(2):
                        nc.sync.dma_start(
                            out=x_sb[:, 1:, t, :], in_=f_r[p, :, 1:, t, :]
                        )
                else:
                    for t in range(2):
                        nc.sync.dma_start(out=x_sb[:, :, t, :], in_=f_r[p, :, :, t, :])

            # HAM warm-up: junk matmuls with no input deps keep the PE at full
            # clock while the first loads are in flight
            junk_sb = consts.tile([128, 512], mybir.dt.bfloat16)
            nc.gpsimd.memset(junk_sb, 0.0)
            junk_ps = pmisc.tile([128, 512], F32, tag="junkp", bufs=1)
            for _ in range(12):
                nc.tensor.matmul(
                    junk_ps, lhsT=junk_sb[:, :128], rhs=junk_sb, start=True, stop=True
                )

            # startup order: x0 arrives in 256KB ch-strips interleaved with
            # wv[mi] slices so the first matmul group streams as data lands
    
            x0 = xpool.tile([128, CH, 2, HW], X_DT, name="x_sb", tag="x_sb")
            xs[0] = x0
            nc.sync.dma_start(out=x0[:, 0:1, :, :], in_=f_r[0, :, 0:1, :, :])
            wv_sb = consts.tile([128, MI, CH, 128], MM_DT)
            nc.sync.dma_start(out=wv_sb[:, 0], in_=wv_r[0])
            nc.sync.dma_start(out=x0[:, 1:2, :, :], in_=f_r[0, :, 1:2, :, :])
            nc.sync.dma_start(out=x0[:, 2:3, :, :], in_=f_r[0, :, 2:3, :, :])
            for t in range(2):
                nc.sync.dma_start(out=x0[:, 3:, t, :], in_=f_r[0, :, 3:, t, :])
            ht_sb = consts.tile([128, KL, BLOC], F32)
            nc.sync.dma_start(
                out=ht_sb, in_=ht_d.rearrange("(kh kl) b -> kl kh b", kl=128)
            )
            wl_sb = consts.tile([128, MI], MM_DT)
            nc.sync.dma_start(
                out=wl_sb, in_=wl_d.rearrange("(mh ml) one -> ml (mh one)", ml=128)
            )
            bvs_sb = consts.tile([128, MI], F32)
            nc.sync.dma_start(
                out=bvs_sb, in_=bvs_d.rearrange("(mh ml) -> ml mh", ml=128)
            )
            ws_sb = consts.tile([128, KL, I], F32)
            ws_r = ws_d.rearrange("(kh kl) i -> kh kl i", kl=128)
            for kh in range(KL):
                nc.sync.dma_start(out=ws_sb[:, kh, :], in_=ws_r[kh])
            for mi in range(1, MI):
                nc.sync.dma_start(out=wv_sb[:, mi], in_=wv_r[mi])
            load_x(1)
            load_x(2)
            load_x(3)
            ident128 = consts.tile([128, 128], F32)
            make_identity(nc, ident128)

            sbias_sb = consts.tile([128, MI, BLOC], F32)
            wsum_sb = outpool.tile([128, CH, BLOC], F32)

            def emit_state():
                # state_out^T = Ws.T @ h^T + (bv + bs); emitted after pair 0's
                # matmuls so the PE doesn't stall on ws/ht at the queue head
                for mi in range(MI):
                    pso = pmisc.tile([128, BLOC], F32, tag="pm")
                    for kh in range(KL):
                        nc.tensor.matmul(
                            pso,
                            lhsT=ws_sb[:, kh, mi * 128 : (mi + 1) * 128],
                            rhs=ht_sb[:, kh, :],
                            start=(kh == 0),
                            stop=(kh == KL - 1),
                        )
                    nc.vector.tensor_scalar_add(
                        sbias_sb[:, mi, :], pso, bvs_sb[:, mi : mi + 1]
                    )

            def process(p, ts, last_unit):
                nt = len(ts)
                # ---- main matmuls ----
                pss = []
                for mi in range(MI):
                    ps = pfeat.tile([128, nt * HW], F32, name="ps", tag="ps")
                    pss.append(ps)
                    for ch in range(CH):
                        rhs = (
                            xs[p][:, ch, :, :]
                            if nt == 2
                            else xs[p][:, ch, ts[0], :]
                        )
                        nc.tensor.matmul(
                            ps,
                            lhsT=wv_sb[:, mi, ch, :],
                            rhs=rhs,
                            start=(ch == 0),
                            stop=(ch == CH - 1),
                        )
                    if p == 0 and mi == 0:
                        emit_state()
                    if p < 3 and nt == 2:
                        for _ in range(3):
                            nc.tensor.matmul(
                                junk_ps,
                                lhsT=junk_sb[:, :128],
                                rhs=junk_sb,
                                start=True,
                                stop=True,
                            )
                if p + 4 < NPAIR and ts[0] == 0:
                    load_x(p + 4)

                # ---- tanh + per-partition state bias ----
                hid_sb = hidpool.tile([128, MI, nt, HW], MM_DT, name="hid_sb", tag="hid")
                for mi in range(MI):
                    for tj, t in enumerate(ts):
                        nc.scalar.activation(
                            out=hid_sb[:, mi, tj, :],
                            in_=pss[mi][:, tj * HW : (tj + 1) * HW],
                            func=Tanh,
                            bias=sbias_sb[:, mi, 2 * p + t : 2 * p + t + 1],
                            scale=1.0,
                        )

                # ---- logits ----
                psl = plg.tile([1, nt * HW], F32, name="psl", tag="psl")
                for mi in range(MI):
                    nc.tensor.matmul(
                        psl,
                        lhsT=wl_sb[:, mi : mi + 1],
                        rhs=hid_sb[:, mi, :, :],
                        start=(mi == 0),
                        stop=(mi == MI - 1),
                    )

                # ---- softmax on partition 0 (logits bounded -> exp is safe
                # without max-subtraction, mathematically identical) ----
                es = smpool.tile([1, nt, HW], F32, tag="es", name="es")
                sums = smpool.tile([1, nt], F32, tag="sums", name="sums")
                if nt == 1:
                    nc.scalar.activation(out=es, in_=psl, func=Exp, accum_out=sums)
                else:
                    nc.scalar.activation(out=es, in_=psl, func=Exp)
                    nc.vector.reduce_sum(out=sums, in_=es, axis=mybir.AxisListType.X)
                rsum = smpool.tile([1, nt], F32, tag="rsum", name="rsum")
                nc.vector.reciprocal(out=rsum, in_=sums)
                scores_st = smpool.tile(
                    [1, nt, HW], F32, tag="scores_st", name="scores_st"
                )
                if nt == 1:
                    nc.vector.tensor_scalar_mul(scores_st, es, rsum)
                else:
                    nc.vector.scalar_tensor_tensor(
                        out=scores_st,
                        in0=es,
                        scalar=1.0,
                        in1=rsum.unsqueeze(2).to_broadcast([1, nt, HW]),
                        op0=MUL,
                        op1=MUL,
                    )
                r0 = 2 * p + ts[0]
                nc.sync.dma_start(out=sout_d[r0 : r0 + nt, :], in_=scores_st)

                # ---- pooling: broadcast scores to all partitions (gpsimd),
                # then fused multiply+accumulate per 128-channel chunk ----
                bcast = bcpool.tile([128, nt, HW], F32, name="bcast", tag="bc")
                nc.gpsimd.partition_broadcast(
                    bcast, scores_st.rearrange("one b s -> one (b s)")
                )
                if last_unit:
                    wT_half = outpool.tile(
                        [8, CH, 128], F32, tag="whalf", bufs=2, name="wT_half"
                    )
                for tj, t in enumerate(ts):
                    i_loc = 2 * p + t
                    for ch in range(CH):
                        sc1 = scratchpool.tile([128, HW], F32, name="sc1")
                        nc.vector.scalar_tensor_tensor(
                            out=sc1,
                            in0=xs[p][:, ch, t, :].bitcast(F32),
                            scalar=1.0,
                            in1=bcast[:, tj, :],
                            op0=MUL,
                            op1=MUL,
                            accum_out=wsum_sb[:, ch, i_loc : i_loc + 1],
                        )
                        if last_unit and t == ts[-1]:
                            # interleave the final per-ch transpose with the
                            # remaining pooling ops (PE is idle here)
                            pst = pmisc.tile([BLOC, 128], F32, tag="pm")
                            nc.tensor.transpose(
                                out=pst[:8, :],
                                in_=wsum_sb[:, ch, 8:16],
                                identity=ident128,
                            )
                            nc.scalar.copy(wT_half[:, ch, :], pst[:8, :])
                if last_unit:
                    nc.sync.dma_start(
                        out=wout_d[8:16, :],
                        in_=wT_half.rearrange("b ch cl -> b (ch cl)"),
                    )

                # ---- first-half weighted transpose mid-kernel ----
                if p == NPAIR // 2 - 1 and ts[-1] == 1:
                    wT_h0 = outpool.tile(
                        [8, CH, 128], F32, tag="whalf", bufs=2, name="wT_h0"
                    )
                    for ch in range(CH):
                        pst = pmisc.tile([BLOC, 128], F32, tag="pm")
                        nc.tensor.transpose(
                            out=pst[:8, :],
                            in_=wsum_sb[:, ch, 0:8],
                            identity=ident128,
                        )
                        nc.scalar.copy(wT_h0[:, ch, :], pst[:8, :])
                    nc.sync.dma_start(
                        out=wout_d[0:8, :],
                        in_=wT_h0.rearrange("b ch cl -> b (ch cl)"),
                    )

            for p in range(NPAIR - 1):
                process(p, (0, 1), False)
            process(NPAIR - 1, (0,), False)
            process(NPAIR - 1, (1,), True)

    nc.compile()
    return nc


_NC = None


def _get_nc():
    global _NC
    if _NC is None:
        _NC = build_nc()
    return _NC


def _round_fp32r(a):
    """Round fp32 to the PE's FP32r format (11-bit mantissa, half-to-even),
    matching walrus's fp32_to_fp32r. Required for tensors consumed by FP32r
    matmuls: the HW expects pre-rounded operands."""
    u = np.ascontiguousarray(a, dtype=np.float32).view(np.uint32)
    lsb = (u >> np.uint32(12)) & np.uint32(1)
    r = (u + np.uint32(0x7FF) + lsb) & np.uint32(0xFFFFF000)
    return r.view(np.float32)


def make_in_maps(features, h_state, Wv, bv, Ws, bs, Wl):
    feats = _round_fp32r(
        np.asarray(features, dtype=np.float32).reshape(B, C, HW)
    )
    hs = np.ascontiguousarray(np.asarray(h_state, dtype=np.float32))
    bvs = (np.asarray(bv, np.float32) + np.asarray(bs, np.float32)).astype(np.float32)
    common = {
        "Wv": _round_fp32r(np.asarray(Wv, dtype=np.float32)),
        "Ws": np.ascontiguousarray(np.asarray(Ws, dtype=np.float32)),
        "Wl": _round_fp32r(np.asarray(Wl, dtype=np.float32)),
        "bvs": bvs,
    }
    return [
        {
            "features": feats[r * BLOC : (r + 1) * BLOC],
            "h_stateT": np.ascontiguousarray(hs[r * BLOC : (r + 1) * BLOC].T),
            **common,
        }
        for r in range(NCORES)
    ]


def kernel(features, h_state, Wv, bv, Ws, bs, Wl, bl):
    from concourse.bass_utils import run_bass_kernel_spmd

    nc = _get_nc()
    in_maps = make_in_maps(features, h_state, Wv, bv, Ws, bs, Wl)
    res = run_bass_kernel_spmd(nc, in_maps, core_ids=list(range(NCORES))).results
    weighted = np.concatenate([res[r]["weighted"] for r in range(NCORES)], axis=0)
    scores = np.concatenate([res[r]["scores"] for r in range(NCORES)], axis=0)
    return weighted, scores.reshape(B, H, W)
